# revision 2
# baseline (speedup 1.0000x reference)
"""Trainium2 Bass kernel for patch attention:
    out = softmax(silu(q) @ silu(k)^T * scale, axis=-1)
with q,k: [B=4, H=16, P=1024, D=128] fp32, scale: [1] fp32.

Sharding: B*H = 64 heads split across 8 NeuronCores, 8 heads each.

The kernel is HBM-bound: 42 MB/core of traffic (33.5 MB output writes)
at ~358 GB/s/core = 117 us floor. Everything is structured to keep the
16 DMA queues saturated from t~0 to the last output byte:

  - nat input DMAs for the first DEPTH heads are emitted BEFORE the
    identity/scale const setup so the sync queue kicks them first.
  - per iteration g: scores/exp/normalize/out-DMA for head g are
    emitted BEFORE the prep (transposes+tanh+stt) of head g+DEPTH, so
    head g+1's scores are not queued behind 16 PE transposes.
  - exp uses ACT accum_out to produce row sums (no DVE tensor_reduce),
    shortening the matmul->out chain; normalize alternates between the
    Pool engine (normalize_recip) and DVE (reciprocal + tensor_scalar)
    so no single engine serializes the per-head epilogue.

Per-core pipeline (per head g), all ACT work in ONE table set
(exp_and_others: tanh + exp), so heads pipeline freely with no
ACT table reloads:
  1. DMA q[g], k[g] as [128, 8, 128] (p-in-tile, p-tile, d) fp32.
  2. PE transpose each [128,128] tile -> PSUM: xT [d=128, p=1024] fp32.
  3. ACT tanh(x/2) (PSUM->SBUF bf16)  [tanh is in the exp table set]
  4. DVE scalar_tensor_tensor: bT = (tanh+1) * xT = 2*silu(x) -> bf16.
     The 2x factors are folded into the softmax scale (scale/4).
  5. Per 128-row p-tile m: two PE matmuls (N=512, bf16) -> scores
     PSUM [128,1024] fp32 (= 4 * silu-scores).
  6. ACT Exp((scale/4)*s) PSUM->SBUF fp32 with accum_out row sums.
  7. Pool normalize_recip (even m) / DVE recip+tensor_scalar (odd m).
  8. DMA out [128, 1024] fp32 rows to HBM.
"""

import numpy as np

B, H, P, D = 4, 16, 1024, 128
N_CORES = 8
G = (B * H) // N_CORES  # heads per core = 8
PT = P // 128  # p-tiles per head = 8

_cached = {}


def _build_module(mm_dtype_name="bfloat16"):
    import concourse.bass as bass
    import concourse.tile as tile
    from concourse import bacc, mybir
    from concourse.masks import make_identity

    f32 = mybir.dt.float32
    mm_dt = getattr(mybir.dt, mm_dtype_name)
    AF = mybir.ActivationFunctionType

    nc = bacc.Bacc("TRN2", target_bir_lowering=False, debug=False)
    q_d = nc.dram_tensor("q", [G, P, D], f32, kind="ExternalInput")
    k_d = nc.dram_tensor("k", [G, P, D], f32, kind="ExternalInput")
    scale_d = nc.dram_tensor("scale", [1], f32, kind="ExternalInput")
    out_d = nc.dram_tensor("out", [G, P, P], f32, kind="ExternalOutput")

    with tile.TileContext(nc) as tc:
        with (
            tc.tile_pool(name="consts", bufs=1) as consts,
            tc.tile_pool(name="nat", bufs=7) as natp,
            tc.tile_pool(name="th", bufs=3) as thp,
            tc.tile_pool(name="bt", bufs=4) as btp,
            tc.tile_pool(name="exp", bufs=4) as expp,
            tc.tile_pool(name="outs", bufs=6) as outp,
            tc.tile_pool(name="stats", bufs=8) as statp,
            tc.tile_pool(name="ps_t", bufs=2, space="PSUM") as ps_tp,
            tc.tile_pool(name="ps_s", bufs=2, space="PSUM") as ps_sp,
        ):
            DEPTH = 3

            def dma_in(g):
                """Kick the q/k HBM reads for head g (no compute deps)."""
                nats = {}
                for nm, src in (("k", k_d), ("q", q_d)):
                    nat = natp.tile([128, PT, 128], f32, tag="nat", name=f"nat_{nm}{g}")
                    nc.sync.dma_start(
                        out=nat, in_=src[g].rearrange("(t p) d -> p t d", p=128)
                    )
                    nats[nm] = nat
                return nats

            # Input DMAs for the first DEPTH heads go FIRST so the sync
            # engine kicks them before any const setup.
            nat_q = [dma_in(g) for g in range(DEPTH)]

            identity = consts.tile([128, 128], f32)
            make_identity(nc, identity)
            scale_sb = consts.tile([128, 1], f32)
            nc.gpsimd.dma_start(out=scale_sb, in_=scale_d[:].to_broadcast([128, 1]))
            # bT = 2*silu => scores are 4x; fold the 1/4 into the exp scale
            scale_adj = consts.tile([128, 1], f32)
            nc.vector.tensor_scalar_mul(scale_adj, scale_sb, 0.25)

            def prep(g, nats):
                """Transposes + tanh + stt for head g -> bf16 bT tiles."""
                bts = {}
                for nm in ("k", "q"):
                    nat = nats[nm]
                    ps_t = ps_tp.tile([128, P], f32, tag="ps_t", name=f"psT_{nm}{g}")
                    for t in range(PT):
                        nc.tensor.transpose(
                            ps_t[:, bass.ts(t, 128)], nat[:, t, :], identity
                        )
                    # tanh(x/2) in the exp_and_others table set
                    th = thp.tile([128, P], mm_dt, tag="th", name=f"th_{nm}{g}")
                    nc.scalar.activation(out=th, in_=ps_t, func=AF.Tanh, scale=0.5)
                    bt = btp.tile([128, P], mm_dt, tag=f"bt_{nm}", name=f"bt_{nm}{g}")
                    nc.vector.scalar_tensor_tensor(
                        out=bt,
                        in0=th,
                        scalar=1.0,
                        in1=ps_t,
                        op0=mybir.AluOpType.add,
                        op1=mybir.AluOpType.mult,
                    )
                    bts[nm] = bt
                return bts["q"], bts["k"]

            ready = [prep(g, nat_q[g]) for g in range(DEPTH)]
            for g in range(G):
                qbT, kbT = ready.pop(0)

                for m in range(PT):
                    ps_s = ps_sp.tile([128, P], f32, tag="ps_s", name=f"psS_{g}_{m}")
                    for h in range(2):
                        nc.tensor.matmul(
                            ps_s[:, bass.ts(h, 512)],
                            qbT[:, bass.ts(m, 128)],
                            kbT[:, bass.ts(h, 512)],
                            start=True,
                            stop=True,
                        )
                    exp_t = expp.tile([128, P], f32, tag="exp", name=f"exp_{g}_{m}")
                    sum_t = statp.tile([128, 1], f32, tag="sum", name=f"sum_{g}_{m}")
                    nc.scalar.activation(
                        out=exp_t,
                        in_=ps_s,
                        func=AF.Exp,
                        scale=scale_adj,
                        accum_out=sum_t,
                    )
                    out_t = outp.tile([128, P], f32, tag="out", name=f"out_{g}_{m}")
                    if m % 2 == 0:
                        nc.gpsimd.normalize_recip(out_t, exp_t, sum_t)
                    else:
                        rec_t = statp.tile(
                            [128, 1], f32, tag="rec", name=f"rec_{g}_{m}"
                        )
                        nc.vector.reciprocal(rec_t, sum_t)
                        nc.vector.tensor_scalar_mul(out_t, exp_t, rec_t)
                    nc.sync.dma_start(
                        out=out_d[g, bass.ts(m, 128), :], in_=out_t
                    )

                if g + DEPTH < G:
                    ready.append(prep(g + DEPTH, dma_in(g + DEPTH)))

    nc.compile()
    return nc


def _get_nc():
    if "nc" not in _cached:
        _cached["nc"] = _build_module()
    return _cached["nc"]


def kernel(q, k, scale, _trace=False):
    from concourse.bass_utils import run_bass_kernel_spmd

    nc = _get_nc()
    qf = np.ascontiguousarray(q.reshape(B * H, P, D), dtype=np.float32)
    kf = np.ascontiguousarray(k.reshape(B * H, P, D), dtype=np.float32)
    sc = np.ascontiguousarray(scale.reshape(1), dtype=np.float32)
    in_maps = [
        {"q": qf[i * G : (i + 1) * G], "k": kf[i * G : (i + 1) * G], "scale": sc}
        for i in range(N_CORES)
    ]
    res = run_bass_kernel_spmd(
        nc, in_maps, core_ids=list(range(N_CORES)), trace=_trace
    )
    out = np.concatenate([res.results[i]["out"] for i in range(N_CORES)], axis=0)
    if _trace:
        kernel.last_result = res
    return out.reshape(B, H, P, P)


# revision 5
# speedup vs baseline: 1.0667x; 1.0667x over previous
"""Trainium2 Bass kernel for patch attention:
    out = softmax(silu(q) @ silu(k)^T * scale, axis=-1)
with q,k: [B=4, H=16, P=1024, D=128] fp32, scale: [1] fp32.

Sharding: B*H = 64 heads split across 8 NeuronCores, 8 heads each.

The kernel is HBM-bound: 42 MB/core of traffic (33.5 MB output writes)
at ~358 GB/s/core = 117 us floor.  Design notes:

* Inputs are loaded CONTIGUOUSLY: q[g] "(p t) d -> p t d" puts rows
  8i..8i+7 on partition i (4 KB contiguous per partition), which the
  HW DGE handles with a cheap descriptor template.  The induced row
  permutation (xT column t*128+j <-> row 8j+t) is absorbed for free:
  - q side: stays permuted; score m-tile m covers rows {8j+m}, and the
    output DMA addresses rows with stride 8 (still 4 KB contiguous
    per partition in HBM).
  - k side: the tanh/stt APs un-permute while reading xT from PSUM
    (strided free-dim access patterns), so kbT is n-natural.
* Output DMA pairs two m-tiles per dma_start: rows 8j+m and 8j+m+1
  are ADJACENT in HBM, so each partition writes 8 KB contiguous.
  4 dma_starts per head instead of 8 halves the sync engine's
  descriptor-writing load (~0.7 us per dma_start).
* exp runs with ACT accum_out (row sums for free); normalization
  alternates between Pool (normalize_recip) and DVE (reciprocal +
  tensor_scalar) so no engine serializes the epilogue.
* PE transposes for head g+2 are interleaved 2-3 per score m-tile of
  head g, and tanh/stt for head g+2 are emitted mid-iteration right
  after their transposes complete.  This keeps ACT (the 12.6 us/head
  pipeline clock: 2 tanh + 8 exp+accum) streaming with no gaps, and
  PE never head-of-line blocks on PSUM transpose buffers.
"""

import numpy as np

B, H, P, D = 4, 16, 1024, 128
N_CORES = 8
G = (B * H) // N_CORES  # heads per core = 8
PT = P // 128  # p-tiles per head = 8

_cached = {}


def _build_module(mm_dtype_name="bfloat16"):
    import concourse.bass as bass
    import concourse.tile as tile
    from concourse import bacc, mybir
    from concourse.masks import make_identity

    f32 = mybir.dt.float32
    mm_dt = getattr(mybir.dt, mm_dtype_name)
    AF = mybir.ActivationFunctionType

    nc = bacc.Bacc("TRN2", target_bir_lowering=False, debug=False)
    q_d = nc.dram_tensor("q", [G, P, D], f32, kind="ExternalInput")
    k_d = nc.dram_tensor("k", [G, P, D], f32, kind="ExternalInput")
    scale_d = nc.dram_tensor("scale", [1], f32, kind="ExternalInput")
    out_d = nc.dram_tensor("out", [G, P, P], f32, kind="ExternalOutput")

    with tile.TileContext(nc) as tc:
        with (
            tc.tile_pool(name="consts", bufs=1) as consts,
            tc.tile_pool(name="nat", bufs=5) as natp,
            tc.tile_pool(name="th", bufs=3) as thp,
            tc.tile_pool(name="bt", bufs=3) as btp,
            tc.tile_pool(name="exp", bufs=3) as expp,
            tc.tile_pool(name="outs", bufs=4) as outp,
            tc.tile_pool(name="stats", bufs=4) as statp,
            tc.tile_pool(name="ps_t", bufs=2, space="PSUM") as ps_tp,
            tc.tile_pool(name="ps_s", bufs=2, space="PSUM") as ps_sp,
        ):
            def dma_in(g):
                """Contiguous q/k loads: partition i <- rows 8i..8i+7."""
                nats = {}
                for nm, src in (("k", k_d), ("q", q_d)):
                    nat = natp.tile([128, PT, 128], f32, tag="nat", name=f"nat_{nm}{g}")
                    nc.sync.dma_start(
                        out=nat, in_=src[g].rearrange("(p t) d -> p t d", t=PT)
                    )
                    nats[nm] = nat
                return nats

            # Input DMAs for the first heads go FIRST so the sync engine
            # kicks them before any const setup.
            nat_pend = {0: dma_in(0), 1: dma_in(1)}

            identity = consts.tile([128, 128], f32)
            make_identity(nc, identity)
            scale_sb = consts.tile([128, 1], f32)
            nc.gpsimd.dma_start(out=scale_sb, in_=scale_d[:].to_broadcast([128, 1]))
            # bT = 2*silu => scores are 4x; fold the 1/4 into the exp scale
            scale_adj = consts.tile([128, 1], f32)
            nc.vector.tensor_scalar_mul(scale_adj, scale_sb, 0.25)

            bts = {}  # g -> (bt_q, bt_k)

            def emit_tanh_stt(g, nm, ps_t):
                """tanh + stt for (head g, tensor nm): bT = 2*silu(x)^T.
                k side un-permutes (column t*128+j -> row 8j+t) via the
                access patterns; q side stays permuted."""
                th = thp.tile([128, P], mm_dt, tag=f"th_{nm}", name=f"th_{nm}{g}")
                bt = btp.tile([128, P], mm_dt, tag=f"bt_{nm}", name=f"bt_{nm}{g}")
                if nm == "k":
                    ps_v = ps_t.rearrange("d (t j) -> d j t", t=PT)
                    th_v = th.rearrange("d (j t) -> d j t", t=PT)
                    bt_v = bt.rearrange("d (j t) -> d j t", t=PT)
                else:
                    ps_v, th_v, bt_v = ps_t, th, bt
                nc.scalar.activation(out=th_v, in_=ps_v, func=AF.Tanh, scale=0.5)
                nc.vector.scalar_tensor_tensor(
                    out=bt_v,
                    in0=th_v,
                    scalar=1.0,
                    in1=ps_v,
                    op0=mybir.AluOpType.add,
                    op1=mybir.AluOpType.mult,
                )
                bts.setdefault(g, {})[nm] = bt

            # Pending transpose micro-ops, popped between score m-tiles.
            pend = []  # list of closures

            def push_transposes(g):
                nats = nat_pend.pop(g)
                for nm in ("k", "q"):
                    nat = nats[nm]
                    ps_t = ps_tp.tile([128, P], f32, tag="ps_t", name=f"psT_{nm}{g}")
                    for t in range(PT):

                        def tr(g=g, nm=nm, nat=nat, ps_t=ps_t, t=t):
                            nc.tensor.transpose(
                                ps_t[:, bass.ts(t, 128)], nat[:, t, :], identity
                            )
                            if t == PT - 1:
                                emit_tanh_stt(g, nm, ps_t)

                        pend.append(tr)

            # Per-m-tile transpose pop counts.  Iteration 0 drains the
            # T(1)+T(2) double push front-loaded (its ps_t slots are free
            # from the pre-loop); steady iterations pop back-weighted so
            # each transpose's PSUM ring slot (freed by the previous
            # head's tanh+stt, ~mid-iteration) is free by pop time.
            POPS_RAMP = [3, 3, 3, 3, 3, 3, 3, 3]
            POPS_STEADY = [0, 0, 2, 2, 3, 3, 3, 3]

            def pop_transposes(g, m):
                n = (POPS_RAMP if g == 0 else POPS_STEADY)[m]
                for _ in range(min(n, len(pend))):
                    pend.pop(0)()

            # Head 0 prep is emitted directly (nothing to interleave with).
            push_transposes(0)
            while pend:
                pend.pop(0)()
            push_transposes(1)

            for g in range(G):
                if g + 2 < G:
                    nat_pend[g + 2] = dma_in(g + 2)
                    push_transposes(g + 2)

                bt_q, bt_k = bts[g]["q"], bts[g]["k"]
                ov = out_d[g].rearrange("(j r) n -> j r n", r=PT)
                out_pair = None
                for m in range(PT):
                    ps_s = ps_sp.tile([128, P], f32, tag="ps_s", name=f"psS_{g}_{m}")
                    for h in range(2):
                        nc.tensor.matmul(
                            ps_s[:, bass.ts(h, 512)],
                            bt_q[:, bass.ts(m, 128)],
                            bt_k[:, bass.ts(h, 512)],
                            start=True,
                            stop=True,
                        )
                    exp_t = expp.tile([128, P], f32, tag="exp", name=f"exp_{g}_{m}")
                    sum_t = statp.tile([128, 1], f32, tag="sum", name=f"sum_{g}_{m}")
                    nc.scalar.activation(
                        out=exp_t,
                        in_=ps_s,
                        func=AF.Exp,
                        scale=scale_adj,
                        accum_out=sum_t,
                    )
                    if m % 2 == 0:
                        out_pair = outp.tile(
                            [128, 2, P], f32, tag="out", name=f"out_{g}_{m // 2}"
                        )
                        nc.gpsimd.normalize_recip(out_pair[:, 0, :], exp_t, sum_t)
                    else:
                        rec_t = statp.tile(
                            [128, 1], f32, tag="rec", name=f"rec_{g}_{m}"
                        )
                        nc.vector.reciprocal(rec_t, sum_t)
                        nc.vector.tensor_scalar_mul(out_pair[:, 1, :], exp_t, rec_t)
                        nc.sync.dma_start(
                            out=ov[:, bass.ts(m // 2, 2), :], in_=out_pair
                        )
                    pop_transposes(g, m)
                while g == G - 1 and pend:
                    pend.pop(0)()

    nc.compile()
    return nc


def _get_nc():
    if "nc" not in _cached:
        _cached["nc"] = _build_module()
    return _cached["nc"]


def kernel(q, k, scale, _trace=False):
    from concourse.bass_utils import run_bass_kernel_spmd

    nc = _get_nc()
    qf = np.ascontiguousarray(q.reshape(B * H, P, D), dtype=np.float32)
    kf = np.ascontiguousarray(k.reshape(B * H, P, D), dtype=np.float32)
    sc = np.ascontiguousarray(scale.reshape(1), dtype=np.float32)
    in_maps = [
        {"q": qf[i * G : (i + 1) * G], "k": kf[i * G : (i + 1) * G], "scale": sc}
        for i in range(N_CORES)
    ]
    res = run_bass_kernel_spmd(
        nc, in_maps, core_ids=list(range(N_CORES)), trace=_trace
    )
    out = np.concatenate([res.results[i]["out"] for i in range(N_CORES)], axis=0)
    if _trace:
        kernel.last_result = res
    return out.reshape(B, H, P, P)
